# revision 3
# baseline (speedup 1.0000x reference)
"""Causal self-attention (B=4, T=2048, C=1024, NH=16) on 8 TRN2 NeuronCores.

Sharding (tensor-parallel heads x data-parallel batch):
  - 4 core-pairs: pair p = cores (2p, 2p+1) handles batch b = p.
  - Within a pair, rank r computes heads 8r..8r+7 (w_qkv output columns
    split by head group) and a 512-column half of the output projection
    (w_proj column split). After attention each core holds attnT [512, T]
    (d-major). Pairwise AllGathers (one per head-pair HALF-strip, bf16)
    exchange these; every core then reads all 8 c_in chunks back from the
    gather output (uniform addressing), so one compiled program serves all
    cores. The host concatenates the column halves.

All matmul operands are bf16 (host converts; rel err ~1e-3); PSUM fp32.

Device program (per core), engineered against the TimelineSim cost model
(PE row budget ~482k rows ~= 201us at 2.4GHz; Act exp ~152us must hide
inside the attention window):
  Phase A  chunk-streamed QKV: wave1 = q,k of pair 0 (8 PSUM banks,
           contraction-chunk outer loop so compute starts when the first
           128-row chunk of xT lands), wave2 = v for t-blocks 0-7.
           Drains on the (otherwise idle) Activation engine; q scaled by
           1/8 in the drain.
  Phase B  attention per head-pair j, q-blocks of 256, kv chunks of 128
           processed in pairs sharing one [128,2,2,256] PSUM tile:
             scores sT[kv,q] (K=64, two heads via PE row groups 0-63/64-127)
             exp on Activation ([128,1024] per chunk-pair)
             causal masking by multiply on the GpSimd/Pool engine
             AV flipped: aT[kv,q] chunks STATIONARY (M=128 q), v_aug
             [kv, 65] moving (N=65) -> aug[q, 65] in PSUM; row 64 is the
             softmax denominator (ones column of v_aug).
           Normalization is fused into the PSUM->SBUF drain (DVE
           tensor_scalar with per-partition reciprocal). Drained
           [q, (qs, 2h*64)] strips are transposed to d-major
           ci[dd, qs, q'] by the XBAR DMA-transpose engine (14ns per
           16x128 tile, no PE/DVE cost). Remaining v t-blocks and q,k of
           pairs 1-3 are interleaved into pairs 0-2 so the PE stays fed
           while Act runs exp.
  Phase C  output projection y[T, 512] over 8 c_in chunks (N=512).
           Half-strip collectives let proj t-blocks 0-7 interleave into
           pair 3's second-half attention; only t-blocks 8-15 trail, with
           the pair-3 chunks reordered last in the accumulation to hide
           the final collective. Drains via Act.
"""

import numpy as np

import concourse.bass as bass
import concourse.mybir as mybir
import concourse.tile as tile
from concourse import bacc
from concourse.bass_utils import run_bass_kernel_spmd

B, T, C = 4, 2048, 1024
NH, HD = 16, 64
N_CORES = 8
HPC = NH // 2          # heads per core
NPAIR = HPC // 2       # head-pairs per core (PE row-group packing unit)
QB = 256               # attention q-block width
NQB = T // QB          # 8 q-blocks per pair
KC = C // 128          # 128-deep contraction chunks for qkv/proj
SCALE = float(1.0 / np.sqrt(HD))

F32 = mybir.dt.float32
BF16 = mybir.dt.bfloat16
AF = mybir.ActivationFunctionType
ALU = mybir.AluOpType
REPLICA_GROUPS = [[0, 1], [2, 3], [4, 5], [6, 7]]


def build_nc(reps=1, single_core=False):
    nc = bacc.Bacc(
        "TRN2", target_bir_lowering=False, debug=False,
        num_devices=(1 if single_core else N_CORES),
    )

    xt = nc.dram_tensor("xt", [C, T], BF16, kind="ExternalInput")
    wq = nc.dram_tensor("wq", [C, 512], BF16, kind="ExternalInput")
    wk = nc.dram_tensor("wk", [C, 512], BF16, kind="ExternalInput")
    wv = nc.dram_tensor("wv", [C, 512], BF16, kind="ExternalInput")
    wp = nc.dram_tensor("wp", [C, 512], BF16, kind="ExternalInput")
    maskA = nc.dram_tensor("maskA", [128, 256], BF16, kind="ExternalInput")
    maskB = nc.dram_tensor("maskB", [128, 512], BF16, kind="ExternalInput")
    y = nc.dram_tensor("y", [T, 512], F32, kind="ExternalOutput")

    with tile.TileContext(nc) as tc:
        for _rep in range(reps):
            _emit_one(nc, tc, xt, wq, wk, wv, wp, maskA, maskB, y,
                      single_core)

    nc.compile()
    return nc


def _emit_one(nc, tc, xt, wq, wk, wv, wp, maskA, maskB, y, single_core):
    with tc.tile_pool(name="persist", bufs=1) as persist, \
         tc.tile_pool(name="ccin", bufs=8, space="DRAM") as ccinp, \
         tc.tile_pool(name="ccout", bufs=8, space="DRAM") as ccoutp:
        xt_sb = persist.tile([128, KC, T], BF16)
        wq_sb = persist.tile([128, KC, 512], BF16)
        wk_sb = persist.tile([128, KC, 512], BF16)
        wv_sb = persist.tile([128, KC, 512], BF16)
        wp_sb = persist.tile([128, KC, 512], BF16)
        qT_sb = persist.tile([128, NPAIR, T], BF16)
        kT_sb = persist.tile([128, NPAIR, T], BF16)
        v_sb = persist.tile([128, T // 128, HPC, HD + 1], BF16)
        mA_sb = persist.tile([128, 2, 128], BF16)
        mB_sb = persist.tile([128, 2, 256], BF16)
        ci = [persist.tile([128, 16, 128], BF16, name=f"ci{j}")
              for j in range(NPAIR)]
        # apf[a]: c_in chunk a = 128*a..128*a+127 of attnT, both halves of T
        apf = [persist.tile([128, 16, 128], BF16, name=f"apf{a}")
               for a in range(KC)]

        xt_r = xt[:].rearrange("(a p) t -> p a t", p=128)
        wq_r = wq[:].rearrange("(a p) n -> p a n", p=128)
        wk_r = wk[:].rearrange("(a p) n -> p a n", p=128)
        wv_r = wv[:].rearrange("(a p) n -> p a n", p=128)
        wp_r = wp[:].rearrange("(a p) n -> p a n", p=128)

        # ---- input DMA stream (ordered for chunk-streamed phase A) ----
        nc.sync.dma_start(out=mA_sb[:], in_=maskA[:].rearrange(
            "p (h q) -> p h q", h=2))
        nc.sync.dma_start(out=mB_sb[:], in_=maskB[:].rearrange(
            "p (h q) -> p h q", h=2))
        for a in range(KC):
            nc.sync.dma_start(out=wq_sb[:, a, :], in_=wq_r[:, a, :])
            nc.sync.dma_start(out=wk_sb[:, a, :], in_=wk_r[:, a, :])
            if a == 0:
                nc.sync.dma_start(out=xt_sb[:, 0, 0:1024],
                                  in_=xt_r[:, 0, 0:1024])
                nc.sync.dma_start(out=xt_sb[:, 0, 1024:T],
                                  in_=xt_r[:, 0, 1024:T])
            else:
                nc.sync.dma_start(out=xt_sb[:, a, :], in_=xt_r[:, a, :])
        for a in range(KC):
            nc.sync.dma_start(out=wv_sb[:, a, :], in_=wv_r[:, a, :])
        for a in range(KC):
            nc.sync.dma_start(out=wp_sb[:, a, :], in_=wp_r[:, a, :])

        # ones column of v_aug (softmax denominators)
        nc.vector.memset(v_sb[:, :, :, HD], 1.0)

        # ---------------- phase A: wave1 qk(pair0), wave2 v(tb0-7) --------
        with tc.tile_pool(name="wave", bufs=8, space="PSUM") as wavep:
            ps_q = [wavep.tile([128, 512], F32, tag="w", name="psq")
                    for _ in range(4)]
            ps_k = [wavep.tile([128, 512], F32, tag="w", name="psk")
                    for _ in range(4)]
            for a in range(KC):
                for b in range(4):
                    nc.tensor.matmul(
                        ps_q[b][:], wq_sb[:, a, 0:128],
                        xt_sb[:, a, 512 * b:512 * (b + 1)],
                        start=(a == 0), stop=(a == KC - 1))
                    nc.tensor.matmul(
                        ps_k[b][:], wk_sb[:, a, 0:128],
                        xt_sb[:, a, 512 * b:512 * (b + 1)],
                        start=(a == 0), stop=(a == KC - 1))
            for b in range(4):
                nc.scalar.activation(qT_sb[:, 0, 512 * b:512 * (b + 1)],
                                     ps_q[b][:], AF.Copy, scale=SCALE)
                nc.scalar.copy(kT_sb[:, 0, 512 * b:512 * (b + 1)], ps_k[b][:])

            ps_v = [wavep.tile([128, 512], F32, tag="w", name="psv")
                    for _ in range(8)]
            for a in range(KC):
                for tb in range(8):
                    nc.tensor.matmul(
                        ps_v[tb][:], xt_sb[:, a, 128 * tb:128 * (tb + 1)],
                        wv_sb[:, a, :],
                        start=(a == 0), stop=(a == KC - 1))
            for tb in range(8):
                nc.scalar.copy(
                    v_sb[:, tb, :, 0:HD],
                    ps_v[tb][:].rearrange("p (h d) -> p h d", h=HPC))

        # ---------------- helpers -----------------------------------------
        def qk_pass(qkp, j, which, b):
            """One sequential q-or-k accumulation pass for pair j, 512-block
            b. In-window drains go on DVE (Act is saturated by exp)."""
            ps = qkp.tile([128, 512], F32, tag="qk", name="qkps")
            wsb = wq_sb if which == "q" else wk_sb
            dst = qT_sb if which == "q" else kT_sb
            for a in range(KC):
                nc.tensor.matmul(
                    ps[:], wsb[:, a, 128 * j:128 * (j + 1)],
                    xt_sb[:, a, 512 * b:512 * (b + 1)],
                    start=(a == 0), stop=(a == KC - 1))
            if which == "q":
                nc.vector.tensor_scalar_mul(
                    dst[:, j, 512 * b:512 * (b + 1)], ps[:], SCALE)
            else:
                nc.vector.tensor_copy(
                    dst[:, j, 512 * b:512 * (b + 1)], ps[:])

        def v_pass(vp, tb):
            ps = vp.tile([128, 512], F32, tag="v", name="vps")
            for a in range(KC):
                nc.tensor.matmul(
                    ps[:], xt_sb[:, a, 128 * tb:128 * (tb + 1)],
                    wv_sb[:, a, :],
                    start=(a == 0), stop=(a == KC - 1))
            nc.vector.tensor_copy(
                v_sb[:, tb, :, 0:HD],
                ps[:].rearrange("p (h d) -> p h d", h=HPC))

        def attn_qb(j, qb, s2p, augp, atp, recp, strip, slot):
            """Attention for pair j, global q-block qb (256 wide). Writes
            normalized bf16 output into strip[:, 2*slot:2*slot+2, :]."""
            q0 = QB * qb
            aug = augp.tile([128, 2, 2, HD + 1], F32, tag="aug", name="aug")
            for cp in range(qb + 1):
                s2 = s2p.tile([128, 2, 2, QB], F32, tag="s2", name="s2")
                for cc in range(2):
                    c = 2 * cp + cc
                    for hh in range(2):
                        nc.tensor.matmul(
                            s2[:, cc, hh, :],
                            kT_sb[64 * hh:64 * hh + 64, j,
                                  128 * c:128 * (c + 1)],
                            qT_sb[64 * hh:64 * hh + 64, j, q0:q0 + QB],
                            start=True, stop=True)
                aT = atp.tile([128, 2, 2, QB], BF16, tag="aT")
                nc.scalar.activation(aT[:], s2[:], AF.Exp)
                if cp == qb:  # diagonal chunk pair: causal masking on Pool
                    nc.gpsimd.tensor_mul(
                        aT[:, 0, :, 0:128], aT[:, 0, :, 0:128], mA_sb[:])
                    nc.gpsimd.tensor_mul(
                        aT[:, 1, :, :], aT[:, 1, :, :], mB_sb[:])
                for cc in range(2):
                    c = 2 * cp + cc
                    diag_odd = (cp == qb and cc == 1)
                    for hh in range(2):
                        for qs in range(2):
                            if diag_odd and qs == 0:
                                continue
                            last = 2 * qb + (1 if qs == 1 else 0)
                            nc.tensor.matmul(
                                aug[:, hh, qs, :],
                                aT[:, cc, hh, 128 * qs:128 * (qs + 1)],
                                v_sb[:, c, 2 * j + hh, :],
                                start=(c == 0), stop=(c == last))
            recip = recp.tile([128, 2, 2, 1], F32, tag="recip")
            nc.vector.reciprocal(recip[:], aug[:, :, :, HD:HD + 1])
            for hh in range(2):
                for qs in range(2):
                    nc.vector.tensor_scalar(
                        strip[:, 2 * slot + qs, 64 * hh:64 * (hh + 1)],
                        aug[:, hh, qs, 0:HD],
                        recip[:, hh, qs, :], None, ALU.mult)

        def fire_collective(j, half):
            """Exchange half-strip (qs 8*half..8*half+7) of ci[j]; load both
            members' rows back into apf chunks (uniform addressing)."""
            ccin = ccinp.tile([128, 1024], BF16, tag="cci", name="cci")
            ccout = ccoutp.tile([256, 1024], BF16, tag="cco", name="cco")
            h0 = 8 * half
            nc.sync.dma_start(
                out=ccin[:],
                in_=ci[j][:, h0:h0 + 8, :].rearrange("p a b -> p (a b)"))
            if single_core:
                nc.sync.dma_start(out=ccout[0:128, :], in_=ccin[:])
                nc.sync.dma_start(out=ccout[128:256, :], in_=ccin[:])
            else:
                nc.gpsimd.collective_compute(
                    "AllGather", ALU.bypass,
                    replica_groups=REPLICA_GROUPS,
                    ins=[ccin.opt()], outs=[ccout.opt()])
            for m in range(2):
                nc.sync.dma_start(
                    out=apf[4 * m + j][:, h0:h0 + 8, :],
                    in_=ccout[128 * m:128 * (m + 1), :].rearrange(
                        "p (a b) -> p a b", b=128))

        def proj_tb(projp, ysbp, tb, a_order):
            """Output projection for t-block tb: y[128tb:128tb+128, 512]."""
            ps = projp.tile([128, 512], F32, tag="pj", name="pjps")
            for i, a in enumerate(a_order):
                nc.tensor.matmul(
                    ps[:], apf[a][:, tb, :], wp_sb[:, a, :],
                    start=(i == 0), stop=(i == KC - 1))
            ysb = ysbp.tile([128, 512], F32, tag="ysb")
            nc.scalar.copy(ysb[:], ps[:])
            nc.sync.dma_start(
                out=y[128 * tb:128 * (tb + 1), :], in_=ysb[:])

        # ---------------- phase B/C ---------------------------------------
        with tc.tile_pool(name="s2", bufs=2, space="PSUM") as s2p, \
             tc.tile_pool(name="aug", bufs=2, space="PSUM") as augp, \
             tc.tile_pool(name="aT", bufs=3) as atp, \
             tc.tile_pool(name="recip", bufs=2) as recp, \
             tc.tile_pool(name="nb", bufs=2) as nbp, \
             tc.tile_pool(name="ysb", bufs=3) as ysbp:

            with tc.tile_pool(name="qkst", bufs=1, space="PSUM") as qkp:
                # pair 0: interleave v tb8-15 + qk p1
                with tc.tile_pool(name="vst", bufs=1, space="PSUM") as vp:
                    j = 0
                    strip = None
                    for qb in range(NQB):
                        if qb % 4 == 0:
                            strip = nbp.tile([128, 8, 128], BF16, tag="nb",
                                             name="nb")
                        attn_qb(j, qb, s2p, augp, atp, recp, strip, qb % 4)
                        v_pass(vp, 8 + qb)
                        qk_pass(qkp, 1, "q" if qb < 4 else "k", qb % 4)
                        if qb % 4 == 3:
                            nc.sync.dma_start_transpose(
                                out=ci[j][:, 2 * qb - 6:2 * qb + 2, :],
                                in_=strip[:])
                            fire_collective(j, qb // 4)
                # pairs 1, 2: interleave qk p2, p3
                for j in (1, 2):
                    strip = None
                    for qb in range(NQB):
                        if qb % 4 == 0:
                            strip = nbp.tile([128, 8, 128], BF16, tag="nb",
                                             name="nb")
                        attn_qb(j, qb, s2p, augp, atp, recp, strip, qb % 4)
                        qk_pass(qkp, j + 1, "q" if qb < 4 else "k", qb % 4)
                        if qb % 4 == 3:
                            nc.sync.dma_start_transpose(
                                out=ci[j][:, 2 * qb - 6:2 * qb + 2, :],
                                in_=strip[:])
                            fire_collective(j, qb // 4)

            # pair 3 first half
            j = 3
            strip = nbp.tile([128, 8, 128], BF16, tag="nb", name="nb")
            for qb in range(4):
                attn_qb(j, qb, s2p, augp, atp, recp, strip, qb)
            nc.sync.dma_start_transpose(out=ci[j][:, 0:8, :], in_=strip[:])
            fire_collective(j, 0)

            # pair 3 second half, proj t-blocks 0-7 interleaved
            with tc.tile_pool(name="proj", bufs=2, space="PSUM") as projp:
                a_nat = list(range(KC))
                strip = nbp.tile([128, 8, 128], BF16, tag="nb", name="nb")
                for m, qb in enumerate(range(4, 8)):
                    attn_qb(j, qb, s2p, augp, atp, recp, strip, m)
                    proj_tb(projp, ysbp, 2 * m, a_nat)
                    proj_tb(projp, ysbp, 2 * m + 1, a_nat)
                nc.sync.dma_start_transpose(out=ci[j][:, 8:16, :],
                                            in_=strip[:])
                fire_collective(j, 1)
                # tail: pair-3 chunks (3, 7) accumulated last to hide the
                # final collective
                a_tail = [0, 1, 2, 4, 5, 6, 3, 7]
                for tb in range(8, 16):
                    proj_tb(projp, ysbp, tb, a_tail)


_NC_CACHE = None


def _get_nc():
    global _NC_CACHE
    if _NC_CACHE is None:
        _NC_CACHE = build_nc()
    return _NC_CACHE


def _masks_np():
    import ml_dtypes
    kv = np.arange(128)[:, None]
    q1 = np.arange(128)[None, :]
    q2 = np.arange(256)[None, :]
    tri = (q1 >= kv).astype(np.float32)                  # chunk 2i, q' 0:128
    triB = (q2 >= kv + 128).astype(np.float32)           # chunk 2i+1
    maskA = np.concatenate([tri, tri], axis=1)           # [128, 256]
    maskB = np.concatenate([triB, triB], axis=1)         # [128, 512]
    return (maskA.astype(ml_dtypes.bfloat16),
            maskB.astype(ml_dtypes.bfloat16))


def shard_inputs(x, w_qkv, w_proj):
    import ml_dtypes
    bf = ml_dtypes.bfloat16
    x = np.asarray(x, dtype=np.float32)
    w_qkv = np.asarray(w_qkv, dtype=np.float32)
    w_proj = np.asarray(w_proj, dtype=np.float32)
    maskA, maskB = _masks_np()
    in_maps = []
    for core in range(N_CORES):
        pair, rank = divmod(core, 2)
        c0 = HD * HPC * rank  # 0 or 512: this core's head-column offset
        in_maps.append({
            "xt": np.ascontiguousarray(x[pair].T).astype(bf),
            "wq": np.ascontiguousarray(w_qkv[:, c0:c0 + 512]).astype(bf),
            "wk": np.ascontiguousarray(
                w_qkv[:, C + c0:C + c0 + 512]).astype(bf),
            "wv": np.ascontiguousarray(
                w_qkv[:, 2 * C + c0:2 * C + c0 + 512]).astype(bf),
            "wp": np.ascontiguousarray(
                w_proj[:, 512 * rank:512 * rank + 512]).astype(bf),
            "maskA": maskA,
            "maskB": maskB,
        })
    return in_maps


def assemble_output(results):
    out = np.empty((B, T, C), dtype=np.float32)
    for core in range(N_CORES):
        pair, rank = divmod(core, 2)
        out[pair][:, 512 * rank:512 * rank + 512] = results[core]["y"]
    return out


# --- cached PJRT runner (same path run_bass_kernel_spmd takes under axon,
# but keeps the jitted executable so repeat calls skip re-tracing) ---
_RUNNER_CACHE = None


def _make_runner(nc):
    import jax
    import numpy as _np
    from jax.sharding import Mesh, PartitionSpec
    from jax.experimental.shard_map import shard_map
    from concourse import bass2jax
    from concourse.bass2jax import _bass_exec_p, install_neuronx_cc_hook

    install_neuronx_cc_hook()
    part_name = (nc.partition_id_tensor.name
                 if nc.partition_id_tensor else None)
    in_names, out_names, out_avals, zero_shapes = [], [], [], []
    for alloc in nc.m.functions[0].allocations:
        if not isinstance(alloc, mybir.MemoryLocationSet):
            continue
        name = alloc.memorylocations[0].name
        if alloc.kind == "ExternalInput":
            if name != part_name:
                in_names.append(name)
        elif alloc.kind == "ExternalOutput":
            out_names.append(name)
            shape = tuple(alloc.tensor_shape)
            dtype = mybir.dt.np(alloc.dtype)
            out_avals.append(jax.core.ShapedArray(shape, dtype))
            zero_shapes.append((shape, dtype))
    n_params = len(in_names)
    n_outs = len(out_names)
    all_in_names = in_names + out_names
    if part_name is not None:
        all_in_names = all_in_names + [part_name]

    def _body(*args):
        operands = list(args)
        if part_name is not None:
            operands.append(bass2jax.partition_id_tensor())
        outs = _bass_exec_p.bind(
            *operands,
            out_avals=tuple(out_avals),
            in_names=tuple(all_in_names),
            out_names=tuple(out_names),
            lowering_input_output_aliases=(),
            sim_require_finite=True,
            sim_require_nnan=True,
            nc=nc,
        )
        return tuple(outs)

    devices = jax.devices()[:N_CORES]
    mesh = Mesh(_np.asarray(devices), ("core",))
    in_specs = (PartitionSpec("core"),) * (n_params + n_outs)
    out_specs = (PartitionSpec("core"),) * n_outs
    donate = tuple(range(n_params, n_params + n_outs))
    sharded = jax.jit(
        shard_map(_body, mesh=mesh, in_specs=in_specs, out_specs=out_specs,
                  check_rep=False),
        donate_argnums=donate, keep_unused=True,
    )

    def run(in_maps):
        concat_in = [
            _np.concatenate([_np.asarray(in_maps[c][nm]) for c in
                             range(N_CORES)], axis=0)
            for nm in in_names
        ]
        concat_zeros = [
            _np.zeros((N_CORES * s[0], *s[1:]), d) for s, d in zero_shapes
        ]
        out_arrs = sharded(*concat_in, *concat_zeros)
        return [
            {nm: _np.asarray(out_arrs[i]).reshape(
                N_CORES, *out_avals[i].shape)[c]
             for i, nm in enumerate(out_names)}
            for c in range(N_CORES)
        ]

    run.sharded = sharded
    run.in_names = in_names
    run.zero_shapes = zero_shapes
    run.mesh = mesh
    return run


def _get_runner():
    global _RUNNER_CACHE
    if _RUNNER_CACHE is None:
        _RUNNER_CACHE = _make_runner(_get_nc())
    return _RUNNER_CACHE


def kernel(x, w_qkv, w_proj):
    in_maps = shard_inputs(x, w_qkv, w_proj)
    try:
        runner = _get_runner()
        # cold-compile executions have produced garbage once before; run
        # twice and only trust agreeing results.
        a1 = assemble_output(runner(in_maps))
        a2 = assemble_output(runner(in_maps))
        if not np.allclose(a1, a2, rtol=1e-3, atol=1e-3):
            a3 = assemble_output(runner(in_maps))
            return a3
        return a2
    except Exception:
        res = run_bass_kernel_spmd(_get_nc(), in_maps, list(range(N_CORES)))
        return assemble_output(res.results)
